# revision 10
# baseline (speedup 1.0000x reference)
"""Causal attention (B=4,T=4096,Dm=1024,Dk=256) on 8 TRN2 NeuronCores.

Sharding: 8 cores = 4 batches x 2 KEY-parity groups. Core (b,h) projects
Q for ALL of batch b's 4096 queries but K/V only for key tiles of parity
h (global key tile g = 2*lt + h). Each query slot j (512 queries) then
attends to exactly its 2j+2 causal parity tiles -- zero padded tiles, and
K/V projection work is split evenly instead of duplicated across the
pair. Softmax is unnormalized exp (no max subtraction), so the two
cores' partial (O, l) simply ADD on the host: O = (O0+O1)/(l0+l1).

Layouts are fully transposed on-chip (Q^T,K^T,S^T,O^T; no PE
transposes). Q^T/K^T are fp8e4m3 so S runs as one DoubleRow matmul per
key tile; V and P stay bf16. The causal mask (incl. each slot's two
diagonal-cut tiles) is data: per-(slot,tile) threshold column and
(col_iota >= thr) * exp(S/16) fused on DVE. O is written back bf16.

v1: software-pipelined schedule. Projection matmuls of column-block cb
are WOVEN with the attention tiles of slots 2(cb-1), 2(cb-1)+1 so the
scalar-engine exp (~690ns/tile, the old attn-phase pacer) hides under
projection tensor work. PSUM is re-banked to 8 exact banks
(K:2, Q/V:1, S/l:3, O:2), l is one ones-matmul per slot off a
full-slot DVE accumulation, output casts/DMAs are per-dv-chunk, and
all output DMAs ride the idle GpSimd queue.
"""

import math
import numpy as np
from contextlib import ExitStack

B, T, DM, DK = 4, 4096, 1024, 256
QW = 512                      # query slot width
NSLOT = 8                     # query slots per core (all of the batch)
NLT = 16                      # local (parity) key tiles per core
CB = 1024                     # xt column-block width (4 blocks)
SCALE = 1.0 / math.sqrt(DK)   # 1/16
# within-slot query permutation of h=1 cores (128-half swap per 256 group;
# involution)
PERM = np.arange(QW).reshape(2, 2, 128)[:, ::-1, :].reshape(QW).copy()

_CACHE = {}


def _build_graph():
    from concourse import bacc, mybir, tile

    f32 = mybir.dt.float32
    bf16 = mybir.dt.bfloat16
    fp8 = mybir.dt.float8e4
    AL = mybir.AluOpType
    EXP = mybir.ActivationFunctionType.Exp
    CPY = mybir.ActivationFunctionType.Copy

    nc = bacc.Bacc(None, target_bir_lowering=False)
    xt = nc.declare_dram_parameter("xt", [DM, T], bf16, isOutput=False)
    # weights host-packed p-major: [128, 8, 256] (contiguous 4KB rows);
    # wk additionally split by dk chunk so the first matmul gates on 256KB
    wq = nc.declare_dram_parameter("wq", [128, 8, DK], bf16, isOutput=False)
    wk0 = nc.declare_dram_parameter("wk0", [128, 8, 128], bf16, isOutput=False)
    wk1 = nc.declare_dram_parameter("wk1", [128, 8, 128], bf16, isOutput=False)
    wv = nc.declare_dram_parameter("wv", [128, 8, DK], bf16, isOutput=False)
    # consts: [bqk(4) | bvb(256) | colio(512) | thr(16)] along free dim
    NCONST = 4 + DK + QW + NLT
    cst = nc.declare_dram_parameter("cst", [128, NCONST], f32, isOutput=False)
    # o packed [slot, part, dvc, q]: one DMA per (slot, dvc), 1KB rows
    o_t = nc.declare_dram_parameter("o_t", [NSLOT, 128, 2, QW], bf16,
                                    isOutput=True)
    l_o = nc.declare_dram_parameter("l_o", [NSLOT, 1, QW], f32, isOutput=True)

    with tile.TileContext(nc) as tc, ExitStack() as ctx:
        const = ctx.enter_context(tc.tile_pool(name="const", bufs=1))
        xt_pool = ctx.enter_context(tc.tile_pool(name="xt_pool", bufs=32))
        kvq = ctx.enter_context(tc.tile_pool(name="kvq", bufs=1))
        p_pool = ctx.enter_context(tc.tile_pool(name="p_pool", bufs=1))
        # PSUM: 8 banks exactly via per-tag bufs in one pool:
        # K accum 2 | Q/V accum 1 | S 2 | l(+warmup) 1 | O 2
        ps_pool = ctx.enter_context(
            tc.tile_pool(name="ps_pool", bufs=1, space="PSUM"))

        # K weights first on the scalar HWDGE queue (they gate the first
        # matmul), then consts (first bias add), then V/Q weights
        wk0_sb = const.tile([128, 8, 128], bf16, tag="wk0")
        wk1_sb = const.tile([128, 8, 128], bf16, tag="wk1")
        wv_sb = const.tile([128, 8, DK], bf16, tag="wv")
        wq_sb = const.tile([128, 8, DK], bf16, tag="wq")
        cst_sb = const.tile([128, NCONST], f32, tag="cst")
        nc.scalar.dma_start(wk0_sb[:], wk0[:])
        nc.scalar.dma_start(wk1_sb[:], wk1[:])
        bqk_sb = cst_sb[:, 0:4]
        bvb_sb = cst_sb[:, 4:4 + DK]
        colio_sb = cst_sb[:, 4 + DK:4 + DK + QW]
        thr_sb = cst_sb[:, 4 + DK + QW:]
        ones_sb = const.tile([128, 1], bf16, tag="ones")
        nc.vector.memset(ones_sb[:], 1.0)
        scratch_sb = const.tile([128, QW], bf16, tag="scratch")
        nc.vector.memset(scratch_sb[:], 0.0)

        # xt tiles as [128, 4, 2, 128]: dim1=x (256-col group), dim2=parity
        # slot, dim3=128 cols. [:, :, h, :] is core h's parity columns.
        xt_cb = {}

        def load_xt(c, cb, eng):
            t_ = xt_pool.tile([128, 4, 2, 128], bf16, tag="xt")
            eng.dma_start(
                t_[:], xt[128 * c:128 * (c + 1),
                          CB * cb:CB * (cb + 1)].rearrange(
                              "p (x par y) -> p x par y", par=2, y=128))
            xt_cb[(c, cb)] = t_

        nc.scalar.dma_start(cst_sb[:], cst[:])
        nc.scalar.dma_start(wq_sb[:], wq[:])
        nc.scalar.dma_start(wv_sb[:], wv[:])
        for cb in range(4):
            for c in range(8):
                load_xt(c, cb, nc.sync)

        # warm-up: dummy matmuls while the first DMAs land -- keeps the PE
        # HAM window busy so real matmuls start at 2.4 GHz, at zero cost
        # (the PE would be idle here anyway)
        warm_ps = ps_pool.tile([1, QW], f32, tag="s", bufs=3,
                               padded_shape=[128, QW])
        for _ in range(8):
            nc.tensor.matmul(warm_ps[:], ones_sb[:], scratch_sb[:],
                             start=True, stop=True)

        kt_sb = kvq.tile([128, 2, NLT * 128], fp8, tag="kt")   # K^T (local)
        vt_sb = kvq.tile([128, NLT, DK], bf16, tag="vt")       # V (local)
        qt_sb = kvq.tile([128, 2, T], fp8, tag="qt")           # Q^T (all)

        # One SPMD graph for both parities: the host pre-swaps xt column
        # pairs for h=1 cores, so [:, :, 0, :] always selects the core's
        # own parity columns (see _prep_inputs).
        def kv_cols(c, cb):
            return xt_cb[(c, cb)][:, :, 0, :]      # [128, 4, 128]

        def proj_units(cb, chunks):
            """Projection work of column block cb as (closure, cost) units.

            Round A holds K dkc0/1 (k-tag, 2 banks) plus the first Q
            sub-accumulation (q-tag); remaining Q sub-rounds (the chunk
            list is per-window: Q chunks are pulled 1-2 windows EARLY so
            attention S/exp can be precomputed ahead of the last window)
            run through the single q bank; round E runs V per key tile."""
            st = {}
            units = []

            def qbias(dkc, chunk):
                nc.vector.tensor_scalar(
                    qt_sb[:, dkc, QW * chunk:QW * (chunk + 1)], st['qa'][:],
                    bqk_sb[:, dkc:dkc + 1], None, AL.add)

            # all (chunk, dkc) sub-rounds; the first is folded into round A
            subs = [(ch, dkc) for ch in chunks for dkc in range(2)]
            first = subs[0] if subs else None
            rest = subs[1:]

            def qmm(c, chunk, dkc, stt, sp):
                # chunk ch reads xt block cb=ch//2, x-groups (ch%2)*2:+2
                qcb = chunk // 2
                x0 = (chunk % 2) * 2
                nc.tensor.matmul(
                    st['qa'][:], wq_sb[:, c, 128 * dkc:128 * (dkc + 1)],
                    xt_cb[(c, qcb)][:, x0:x0 + 2, :, :], start=stt, stop=sp)

            for c in range(8):
                def f(c=c):
                    if c == 0:
                        st['k0'] = ps_pool.tile([128, QW], f32, tag="k",
                                                bufs=2, name="kps0")
                        st['k1'] = ps_pool.tile([128, QW], f32, tag="k",
                                                bufs=2, name="kps1")
                        if first:
                            st['qa'] = ps_pool.tile([128, QW], f32, tag="q",
                                                    bufs=1, name="qacc")
                    stt, sp = (c == 0), (c == 7)
                    nc.tensor.matmul(st['k0'][:], wk0_sb[:, c, :],
                                     kv_cols(c, cb), start=stt, stop=sp)
                    nc.tensor.matmul(st['k1'][:], wk1_sb[:, c, :],
                                     kv_cols(c, cb), start=stt, stop=sp)
                    if first:
                        qmm(c, first[0], first[1], stt, sp)
                    if c == 7:
                        for dkc, ps in ((0, st['k0']), (1, st['k1'])):
                            nc.vector.tensor_scalar(
                                kt_sb[:, dkc, QW * cb:QW * (cb + 1)], ps[:],
                                bqk_sb[:, 2 + dkc:3 + dkc], None, AL.add)
                        if first:
                            qbias(first[1], first[0])
                units.append((f, 648 if first else 432))
            for chunk, dkc in rest:
                for c in range(8):
                    def f(c=c, dkc=dkc, chunk=chunk):
                        if c == 0:
                            st['qa'] = ps_pool.tile(
                                [128, QW], f32, tag="q", bufs=1, name="qacc")
                        qmm(c, chunk, dkc, c == 0, c == 7)
                        if c == 7:
                            qbias(dkc, chunk)
                    units.append((f, 216))
            for x in range(4):
                lt = 4 * cb + x
                for c in range(8):
                    def f(c=c, x=x, lt=lt):
                        if c == 0:
                            st['v'] = ps_pool.tile(
                                [128, QW], f32, tag="q", bufs=1, name="vps")
                        nc.tensor.matmul(
                            st['v'][:, :DK], xt_cb[(c, cb)][:, x, 0, :],
                            wv_sb[:, c, :], start=(c == 0), stop=(c == 7))
                        if c == 7:
                            nc.vector.tensor_tensor(
                                vt_sb[:, lt, :], st['v'][:, :DK], bvb_sb[:],
                                AL.add)
                    units.append((f, 108))
            return units

        def make_slot(j, o_tag="o", out_eng=None):
            """Attention slot j split into pre_units (S+exp+l-acc for tiles
            whose kt/qt are already live -- emitted 1-2 windows early, P
            buffered in SBUF) and post_units (remaining S plus ALL O pairs
            and the epilogue). S matmuls go in bursts of 3 (the s-tag PSUM
            depth): a DoubleRow matmul only streams at 1 col/cycle right
            after another DR matmul, so bursts amortize the bf16->DR
            mode-switch tax. Diagonal-cut (masked) tiles are ordered so the
            final tile's exp->O chain avoids the DVE mask hop when
            possible."""
            E = 2 * j + 2
            st = {}
            oe = out_eng or [nc.gpsimd, nc.gpsimd, nc.gpsimd]

            def emit_s(lt):
                s_ps = ps_pool.tile([128, QW], f32, tag="s", bufs=3)
                nc.tensor.matmul(
                    s_ps[:],
                    kt_sb[:, 0:2, 128 * lt:128 * (lt + 1)],
                    qt_sb[:, 0:2, QW * j:QW * (j + 1)],
                    start=True, stop=True,
                    perf_mode=mybir.MatmulPerfMode.DoubleRow)
                p_raw = p_pool.tile([128, QW], bf16, tag="praw", bufs=36)
                nc.scalar.activation(p_raw[:], s_ps[:], EXP, scale=SCALE)
                if lt >= E - 2:
                    # only the last two parity tiles can be diagonal-cut
                    p_m = p_pool.tile([128, QW], bf16, tag="pm", bufs=6)
                    nc.vector.scalar_tensor_tensor(
                        p_m[:], colio_sb[:], thr_sb[:, lt:lt + 1],
                        p_raw[:], AL.is_ge, AL.mult)
                else:
                    p_m = p_raw
                st[lt] = p_m
                # l accumulates on DVE; one ones-matmul per slot at the end
                if 'pacc' not in st:
                    st['pacc'] = p_pool.tile([128, QW], bf16, tag="pacc",
                                             name="pacc", bufs=5)
                    nc.vector.tensor_copy(st['pacc'][:], p_m[:])
                else:
                    nc.vector.tensor_tensor(
                        st['pacc'][:], st['pacc'][:], p_m[:], AL.add)

            def s_burst_units(tiles):
                units = []
                for i in range(0, len(tiles), 3):
                    g = tiles[i:i + 3]

                    def f(g=g):
                        for lt in g:
                            emit_s(lt)
                    units.append((f, 216 * len(g) + 350))
                return units

            def pre_units(tiles):
                return s_burst_units(tiles)

            def post_units(fresh, o_order):
                units = s_burst_units(fresh)

                def mk_o(k, dvc):
                    lt = o_order[k]

                    def f():
                        if k == 0 and dvc == 0:
                            st['o'] = [
                                ps_pool.tile([128, QW], f32, tag=o_tag,
                                             bufs=2, name="ops0"),
                                ps_pool.tile([128, QW], f32, tag=o_tag,
                                             bufs=2, name="ops1")]
                        nc.tensor.matmul(
                            st['o'][dvc][:],
                            vt_sb[:, lt, 128 * dvc:128 * (dvc + 1)],
                            st[lt][:],
                            start=(k == 0), stop=(k == len(o_order) - 1))
                    return f

                for k in range(len(o_order)):
                    units.append((mk_o(k, 0), 216))
                    units.append((mk_o(k, 1), 216))

                def fin():
                    l_ps = ps_pool.tile([1, QW], f32, tag="s", bufs=3,
                                        padded_shape=[128, QW])
                    nc.tensor.matmul(l_ps[:], ones_sb[:], st['pacc'][:],
                                     start=True, stop=True)
                    for dvc in range(2):
                        osb = p_pool.tile([128, QW], bf16, tag="osb", bufs=4)
                        nc.vector.tensor_copy(osb[:], st['o'][dvc][:])
                        oe[dvc].dma_start(o_t[j][:, dvc, :], osb[:])
                    l_sb = p_pool.tile([1, QW], f32, tag="lsb", bufs=3)
                    nc.scalar.activation(l_sb[:], l_ps[:], CPY)
                    oe[2].dma_start(l_o[j], l_sb[:])
                units.append((fin, 400))
                return units

            return pre_units, post_units

        def weave(pu, au):
            """Emit attn units at even tensor-time intervals through the
            proj unit stream (both are (closure, cost) lists)."""
            if not au:
                for f, _ in pu:
                    f()
                return
            pt = sum(c for _, c in pu)
            at = sum(c for _, c in au)
            pi = 0
            spent_p = 0.0
            spent_a = 0.0
            for f, c in au:
                # emit proj until its progress fraction catches attn's
                while pi < len(pu) and spent_p * at <= spent_a * pt:
                    pf, pc = pu[pi]
                    pf()
                    spent_p += pc
                    pi += 1
                f()
                spent_a += c
            while pi < len(pu):
                pu[pi][0]()
                pi += 1

        # slots: E_j = 2j+2 tiles; tile lt needs kt from window lt//4 and
        # qt chunk j. Q chunks are computed early (see table below) so
        # slots 4-7 can precompute S+exp 1-2 windows ahead, leaving the
        # final window as a dense bf16 O-stream instead of a serial
        # exp-bound phase.
        QCH = {0: [0, 1], 1: [2, 3, 4, 5], 2: [6, 7], 3: []}
        slot_fns = {}
        for j in range(8):
            o_tag = "k" if j == 7 else "o"
            out_eng = [nc.gpsimd, nc.sync, nc.scalar] if j == 6 else None
            slot_fns[j] = make_slot(j, o_tag=o_tag, out_eng=out_eng)

        def full(j):
            pre, post = slot_fns[j]
            E = 2 * j + 2
            order = ([E - 2, E - 1] + list(range(E - 2))) if E > 2 \
                else list(range(E))
            return post(order, order)

        weave(proj_units(0, QCH[0]), [])
        weave(proj_units(1, QCH[1]), full(0) + full(1))
        # windows 2-3: current slots plus S/exp precompute for later slots
        pre4, post4 = slot_fns[4]
        pre5, post5 = slot_fns[5]
        pre6, post6 = slot_fns[6]
        pre7, post7 = slot_fns[7]
        weave(proj_units(2, QCH[2]),
              full(2) + full(3) + pre4(list(range(8))) + pre5(list(range(8))))
        # slot 4 (E=10): fresh 8,9 are the diag pair; slot 5 (E=12): fresh
        # 8..11 with diag 10,11 first so the final tile is unmasked
        weave(proj_units(3, QCH[3]),
              post4([8, 9], [8, 9] + list(range(8)))
              + post5([10, 11, 8, 9], [10, 11, 8, 9] + list(range(8)))
              + pre7(list(range(12))) + pre6(list(range(12))))
        # last window: slot 7 completes (and writes out) first, then 6
        for fu, _ in post7([14, 15, 12, 13], [14, 15, 12, 13]
                           + list(range(12))):
            fu()
        for fu, _ in post6([12, 13], [12, 13] + list(range(12))):
            fu()

    nc.compile()
    return nc


def _prep_inputs(inputs, Wq, bq, Wk, bk, Wv, bv):
    import ml_dtypes
    bf16 = ml_dtypes.bfloat16

    def pack_w(W):
        return np.ascontiguousarray(
            W.reshape(8, 128, DK).transpose(1, 0, 2)).astype(bf16)

    wq_, wk_, wv_ = pack_w(Wq), pack_w(Wk), pack_w(Wv)
    wk0_ = np.ascontiguousarray(wk_[:, :, :128])
    wk1_ = np.ascontiguousarray(wk_[:, :, 128:])
    bqk = np.stack([bq[:128], bq[128:], bk[:128], bk[128:]],
                   axis=1).astype(np.float32)
    bvb = np.tile(bv[None, :], (128, 1)).astype(np.float32)

    # xt per (b, h): h=1 cores get each 256-col group's halves swapped so
    # the kernel's fixed [:, :, 0, :] parity read picks the odd tiles.
    # Side effect: h=1 query columns are pair-permuted within each slot;
    # colio then carries the ORIGINAL query index per position (for the
    # causal mask) and the host unswaps O/l columns before summing.
    perm = PERM
    xt_b = [np.ascontiguousarray(inputs[b].T).astype(bf16) for b in range(B)]
    xt_bh = {}
    for b in range(B):
        xt_bh[(b, 0)] = xt_b[b]
        sw = xt_b[b].reshape(DM, T // 256, 2, 128)[:, :, ::-1, :]
        xt_bh[(b, 1)] = np.ascontiguousarray(sw.reshape(DM, T))

    in_maps = []
    r = np.arange(128, dtype=np.float32)
    for core in range(8):
        b, h = core % B, core // B
        cvals = np.arange(QW, dtype=np.float32) if h == 0 \
            else perm.astype(np.float32)
        colio = np.tile(cvals[None, :], (128, 1))
        thr_np = np.empty((128, NLT), dtype=np.float32)
        for lt in range(NLT):
            j = lt // 2
            g = 2 * lt + h
            thr_np[:, lt] = 128 * g + r - QW * j
        cst = np.concatenate([bqk, bvb, colio, thr_np], axis=1)
        in_maps.append({
            "xt": xt_bh[(b, h)],
            "wq": wq_, "wk0": wk0_, "wk1": wk1_, "wv": wv_,
            "cst": np.ascontiguousarray(cst),
        })
    return in_maps


def kernel(inputs, Wq, bq, Wk, bk, Wv, bv):
    from concourse.bass_utils import run_bass_kernel_spmd

    if "nc" not in _CACHE:
        _CACHE["nc"] = _build_graph()
    nc = _CACHE["nc"]

    in_maps = _prep_inputs(
        np.asarray(inputs), np.asarray(Wq), np.asarray(bq), np.asarray(Wk),
        np.asarray(bk), np.asarray(Wv), np.asarray(bv))

    res = run_bass_kernel_spmd(nc, in_maps, core_ids=list(range(8)))
    _CACHE["last_results"] = res

    out = np.empty((B, T, DK), dtype=np.float32)
    for b in range(B):
        r0, r1 = res.results[b], res.results[b + 4]

        def unpack(r):
            # [slot, part, dvc, q] -> [slot, dv, q]
            o = np.asarray(r["o_t"]).astype(np.float32)
            return o.transpose(0, 2, 1, 3).reshape(NSLOT, DK, QW)

        o_sum = unpack(r0) + unpack(r1)[:, :, PERM]
        l_sum = np.asarray(r0["l_o"]) \
            + np.asarray(r1["l_o"])[:, :, PERM]
        for j in range(NSLOT):
            out[b, QW * j:QW * (j + 1), :] = (o_sum[j] / l_sum[j]).T
    return out


if __name__ == "__main__":
    import reference
    ins = {k: np.asarray(v) for k, v in reference.setup_inputs().items()}
    exp = np.asarray(reference.reference(**ins))
    act = kernel(**ins)
    err = np.linalg.norm(act - exp) / np.linalg.norm(exp)
    print("Relative error:", err)


# revision 11
# speedup vs baseline: 1.1139x; 1.1139x over previous
"""Causal attention (B=4,T=4096,Dm=1024,Dk=256) on 8 TRN2 NeuronCores.

Sharding: 8 cores = 4 batches x 2 KEY-parity groups. Core (b,h) projects
Q for ALL of batch b's 4096 queries but K/V only for key tiles of parity
h (global key tile g = 2*lt + h). Each query slot j (512 queries) then
attends to exactly its 2j+2 causal parity tiles -- zero padded tiles, and
K/V projection work is split evenly instead of duplicated across the
pair. Softmax is unnormalized exp (no max subtraction), so the two
cores' partial (O, l) simply ADD on the host: O = (O0+O1)/(l0+l1).

Layouts are fully transposed on-chip (Q^T,K^T,S^T,O^T; no PE
transposes). Q^T/K^T are fp8e4m3 so S runs as one DoubleRow matmul per
key tile; V and P stay bf16. The causal mask (incl. each slot's two
diagonal-cut tiles) is data: per-(slot,tile) threshold column and
(col_iota >= thr) * exp(S/16) fused on DVE. O is written back bf16.

v1: software-pipelined schedule. Projection matmuls of column-block cb
are WOVEN with the attention tiles of slots 2(cb-1), 2(cb-1)+1 so the
scalar-engine exp (~690ns/tile, the old attn-phase pacer) hides under
projection tensor work. PSUM is re-banked to 8 exact banks
(K:2, Q/V:1, S/l:3, O:2), l is one ones-matmul per slot off a
full-slot DVE accumulation, output casts/DMAs are per-dv-chunk, and
all output DMAs ride the idle GpSimd queue.
"""

import math
import numpy as np
from contextlib import ExitStack

B, T, DM, DK = 4, 4096, 1024, 256
QW = 512                      # query slot width
NSLOT = 8                     # query slots per core (all of the batch)
NLT = 16                      # local (parity) key tiles per core
CB = 1024                     # xt column-block width (4 blocks)
SCALE = 1.0 / math.sqrt(DK)   # 1/16
# within-slot query permutation of h=1 cores (128-half swap per 256 group;
# involution)
PERM = np.arange(QW).reshape(2, 2, 128)[:, ::-1, :].reshape(QW).copy()

_CACHE = {}


def _build_graph():
    from concourse import bacc, mybir, tile

    f32 = mybir.dt.float32
    bf16 = mybir.dt.bfloat16
    fp8 = mybir.dt.float8e4
    AL = mybir.AluOpType
    EXP = mybir.ActivationFunctionType.Exp
    CPY = mybir.ActivationFunctionType.Copy

    nc = bacc.Bacc(None, target_bir_lowering=False)
    xt = nc.declare_dram_parameter("xt", [DM, T], bf16, isOutput=False)
    # weights host-packed p-major: [128, 8, 256] (contiguous 4KB rows);
    # wk additionally split by dk chunk so the first matmul gates on 256KB
    wq = nc.declare_dram_parameter("wq", [128, 8, DK], bf16, isOutput=False)
    wk0 = nc.declare_dram_parameter("wk0", [128, 8, 128], bf16, isOutput=False)
    wk1 = nc.declare_dram_parameter("wk1", [128, 8, 128], bf16, isOutput=False)
    wv = nc.declare_dram_parameter("wv", [128, 8, DK], bf16, isOutput=False)
    # consts: [bqk(4) | bvb(256) | colio(512) | thr(16)] along free dim
    NCONST = 4 + DK + QW + NLT
    cst = nc.declare_dram_parameter("cst", [128, NCONST], f32, isOutput=False)
    # o packed [slot, part, dvc, q]: one DMA per (slot, dvc), 1KB rows
    o_t = nc.declare_dram_parameter("o_t", [NSLOT, 128, 2, QW], bf16,
                                    isOutput=True)
    l_o = nc.declare_dram_parameter("l_o", [NSLOT, 1, QW], f32, isOutput=True)

    with tile.TileContext(nc) as tc, ExitStack() as ctx:
        const = ctx.enter_context(tc.tile_pool(name="const", bufs=1))
        xt_pool = ctx.enter_context(tc.tile_pool(name="xt_pool", bufs=32))
        kvq = ctx.enter_context(tc.tile_pool(name="kvq", bufs=1))
        p_pool = ctx.enter_context(tc.tile_pool(name="p_pool", bufs=1))
        # PSUM: 8 banks exactly via per-tag bufs in one pool:
        # K accum 2 | Q/V accum 1 | S 2 | l(+warmup) 1 | O 2
        ps_pool = ctx.enter_context(
            tc.tile_pool(name="ps_pool", bufs=1, space="PSUM"))

        # K weights first on the scalar HWDGE queue (they gate the first
        # matmul), then consts (first bias add), then V/Q weights
        wk0_sb = const.tile([128, 8, 128], bf16, tag="wk0")
        wk1_sb = const.tile([128, 8, 128], bf16, tag="wk1")
        wv_sb = const.tile([128, 8, DK], bf16, tag="wv")
        wq_sb = const.tile([128, 8, DK], bf16, tag="wq")
        cst_sb = const.tile([128, NCONST], f32, tag="cst")
        nc.scalar.dma_start(wk0_sb[:], wk0[:])
        nc.scalar.dma_start(wk1_sb[:], wk1[:])
        bqk_sb = cst_sb[:, 0:4]
        bvb_sb = cst_sb[:, 4:4 + DK]
        colio_sb = cst_sb[:, 4 + DK:4 + DK + QW]
        thr_sb = cst_sb[:, 4 + DK + QW:]
        ones_sb = const.tile([128, 1], bf16, tag="ones")
        nc.vector.memset(ones_sb[:], 1.0)
        scratch_sb = const.tile([128, QW], bf16, tag="scratch")
        nc.vector.memset(scratch_sb[:], 0.0)

        # xt tiles as [128, 4, 2, 128]: dim1=x (256-col group), dim2=parity
        # slot, dim3=128 cols. [:, :, h, :] is core h's parity columns.
        xt_cb = {}

        def load_xt(c, cb, eng):
            t_ = xt_pool.tile([128, 4, 2, 128], bf16, tag="xt")
            eng.dma_start(
                t_[:], xt[128 * c:128 * (c + 1),
                          CB * cb:CB * (cb + 1)].rearrange(
                              "p (x par y) -> p x par y", par=2, y=128))
            xt_cb[(c, cb)] = t_

        nc.scalar.dma_start(cst_sb[:], cst[:])
        nc.scalar.dma_start(wq_sb[:], wq[:])
        nc.scalar.dma_start(wv_sb[:], wv[:])
        for cb in range(4):
            for c in range(8):
                load_xt(c, cb, nc.sync)

        # warm-up: dummy matmuls while the first DMAs land -- keeps the PE
        # HAM window busy so real matmuls start at 2.4 GHz, at zero cost
        # (the PE would be idle here anyway)
        warm_ps = ps_pool.tile([1, QW], f32, tag="s", bufs=3,
                               padded_shape=[128, QW])
        for _ in range(8):
            nc.tensor.matmul(warm_ps[:], ones_sb[:], scratch_sb[:],
                             start=True, stop=True)

        kt_sb = kvq.tile([128, 2, NLT * 128], fp8, tag="kt")   # K^T (local)
        vt_sb = kvq.tile([128, NLT, DK], bf16, tag="vt")       # V (local)
        qt_sb = kvq.tile([128, 2, T], fp8, tag="qt")           # Q^T (all)

        # One SPMD graph for both parities: the host pre-swaps xt column
        # pairs for h=1 cores, so [:, :, 0, :] always selects the core's
        # own parity columns (see _prep_inputs).
        def kv_cols(c, cb):
            return xt_cb[(c, cb)][:, :, 0, :]      # [128, 4, 128]

        def proj_units(cb, chunks):
            """Projection work of column block cb as (closure, cost) units.

            Round A holds K dkc0/1 (k-tag, 2 banks) plus the first Q
            sub-accumulation (q-tag); remaining Q sub-rounds (the chunk
            list is per-window: Q chunks are pulled 1-2 windows EARLY so
            attention S/exp can be precomputed ahead of the last window)
            run through the single q bank; round E runs V per key tile."""
            st = {}
            units = []

            def qbias(dkc, chunk):
                nc.vector.tensor_scalar(
                    qt_sb[:, dkc, QW * chunk:QW * (chunk + 1)], st['qa'][:],
                    bqk_sb[:, dkc:dkc + 1], None, AL.add)

            # all (chunk, dkc) sub-rounds; the first is folded into round A
            subs = [(ch, dkc) for ch in chunks for dkc in range(2)]
            first = subs[0] if subs else None
            rest = subs[1:]

            def qmm(c, chunk, dkc, stt, sp):
                # chunk ch reads xt block cb=ch//2, x-groups (ch%2)*2:+2
                qcb = chunk // 2
                x0 = (chunk % 2) * 2
                nc.tensor.matmul(
                    st['qa'][:], wq_sb[:, c, 128 * dkc:128 * (dkc + 1)],
                    xt_cb[(c, qcb)][:, x0:x0 + 2, :, :], start=stt, stop=sp)

            for c in range(8):
                def f(c=c):
                    if c == 0:
                        st['k0'] = ps_pool.tile([128, QW], f32, tag="k",
                                                bufs=2, name="kps0")
                        st['k1'] = ps_pool.tile([128, QW], f32, tag="k",
                                                bufs=2, name="kps1")
                        if first:
                            st['qa'] = ps_pool.tile([128, QW], f32, tag="q",
                                                    bufs=1, name="qacc")
                    stt, sp = (c == 0), (c == 7)
                    nc.tensor.matmul(st['k0'][:], wk0_sb[:, c, :],
                                     kv_cols(c, cb), start=stt, stop=sp)
                    nc.tensor.matmul(st['k1'][:], wk1_sb[:, c, :],
                                     kv_cols(c, cb), start=stt, stop=sp)
                    if first:
                        qmm(c, first[0], first[1], stt, sp)
                    if c == 7:
                        for dkc, ps in ((0, st['k0']), (1, st['k1'])):
                            nc.vector.tensor_scalar(
                                kt_sb[:, dkc, QW * cb:QW * (cb + 1)], ps[:],
                                bqk_sb[:, 2 + dkc:3 + dkc], None, AL.add)
                        if first:
                            qbias(first[1], first[0])
                units.append((f, 648 if first else 432))
            for chunk, dkc in rest:
                for c in range(8):
                    def f(c=c, dkc=dkc, chunk=chunk):
                        if c == 0:
                            st['qa'] = ps_pool.tile(
                                [128, QW], f32, tag="q", bufs=1, name="qacc")
                        qmm(c, chunk, dkc, c == 0, c == 7)
                        if c == 7:
                            qbias(dkc, chunk)
                    units.append((f, 216))
            for x in range(4):
                lt = 4 * cb + x
                for c in range(8):
                    def f(c=c, x=x, lt=lt):
                        if c == 0:
                            st['v'] = ps_pool.tile(
                                [128, QW], f32, tag="q", bufs=1, name="vps")
                        nc.tensor.matmul(
                            st['v'][:, :DK], xt_cb[(c, cb)][:, x, 0, :],
                            wv_sb[:, c, :], start=(c == 0), stop=(c == 7))
                        if c == 7:
                            nc.vector.tensor_tensor(
                                vt_sb[:, lt, :], st['v'][:, :DK], bvb_sb[:],
                                AL.add)
                    units.append((f, 108))
            return units

        def make_slot(j, o_tag="o", out_eng=None):
            """Attention slot j split into pre_units (S+exp+l-acc for tiles
            whose kt/qt are already live -- emitted 1-2 windows early, P
            buffered in SBUF) and post_units (remaining S plus ALL O pairs
            and the epilogue). S matmuls go in bursts of 3 (the s-tag PSUM
            depth): a DoubleRow matmul only streams at 1 col/cycle right
            after another DR matmul, so bursts amortize the bf16->DR
            mode-switch tax. Diagonal-cut (masked) tiles are ordered so the
            final tile's exp->O chain avoids the DVE mask hop when
            possible."""
            E = 2 * j + 2
            st = {}
            oe = out_eng or [nc.gpsimd, nc.gpsimd, nc.gpsimd]

            def emit_s(lt):
                s_ps = ps_pool.tile([128, QW], f32, tag="s", bufs=3)
                nc.tensor.matmul(
                    s_ps[:],
                    kt_sb[:, 0:2, 128 * lt:128 * (lt + 1)],
                    qt_sb[:, 0:2, QW * j:QW * (j + 1)],
                    start=True, stop=True,
                    perf_mode=mybir.MatmulPerfMode.DoubleRow)
                p_raw = p_pool.tile([128, QW], bf16, tag="praw", bufs=36)
                nc.scalar.activation(p_raw[:], s_ps[:], EXP, scale=SCALE)
                if lt >= E - 2:
                    # only the last two parity tiles can be diagonal-cut
                    p_m = p_pool.tile([128, QW], bf16, tag="pm", bufs=6)
                    nc.vector.scalar_tensor_tensor(
                        p_m[:], colio_sb[:], thr_sb[:, lt:lt + 1],
                        p_raw[:], AL.is_ge, AL.mult)
                else:
                    p_m = p_raw
                st[lt] = p_m
                # l accumulates on DVE; one ones-matmul per slot at the end
                if 'pacc' not in st:
                    st['pacc'] = p_pool.tile([128, QW], bf16, tag="pacc",
                                             name="pacc", bufs=5)
                    nc.vector.tensor_copy(st['pacc'][:], p_m[:])
                else:
                    nc.vector.tensor_tensor(
                        st['pacc'][:], st['pacc'][:], p_m[:], AL.add)

            def s_burst_units(tiles):
                units = []
                for i in range(0, len(tiles), 3):
                    g = tiles[i:i + 3]

                    def f(g=g):
                        for lt in g:
                            emit_s(lt)
                    units.append((f, 216 * len(g) + 350))
                return units

            def pre_units(tiles):
                return s_burst_units(tiles)

            def post_units(fresh, o_order):
                units = s_burst_units(fresh)

                def mk_o(k, dvc):
                    lt = o_order[k]

                    def f():
                        if k == 0 and dvc == 0:
                            st['o'] = [
                                ps_pool.tile([128, QW], f32, tag=o_tag,
                                             bufs=2, name="ops0"),
                                ps_pool.tile([128, QW], f32, tag=o_tag,
                                             bufs=2, name="ops1")]
                        nc.tensor.matmul(
                            st['o'][dvc][:],
                            vt_sb[:, lt, 128 * dvc:128 * (dvc + 1)],
                            st[lt][:],
                            start=(k == 0), stop=(k == len(o_order) - 1))
                    return f

                for k in range(len(o_order)):
                    units.append((mk_o(k, 0), 216))
                    units.append((mk_o(k, 1), 216))

                def fin():
                    l_ps = ps_pool.tile([1, QW], f32, tag="s", bufs=3,
                                        padded_shape=[128, QW])
                    nc.tensor.matmul(l_ps[:], ones_sb[:], st['pacc'][:],
                                     start=True, stop=True)
                    for dvc in range(2):
                        osb = p_pool.tile([128, QW], bf16, tag="osb", bufs=4)
                        nc.vector.tensor_copy(osb[:], st['o'][dvc][:])
                        oe[dvc].dma_start(o_t[j][:, dvc, :], osb[:])
                    l_sb = p_pool.tile([1, QW], f32, tag="lsb", bufs=3)
                    nc.scalar.activation(l_sb[:], l_ps[:], CPY)
                    oe[2].dma_start(l_o[j], l_sb[:])
                units.append((fin, 400))
                return units

            return pre_units, post_units

        def weave(pu, au):
            """Emit attn units at even tensor-time intervals through the
            proj unit stream (both are (closure, cost) lists)."""
            if not au:
                for f, _ in pu:
                    f()
                return
            pt = sum(c for _, c in pu)
            at = sum(c for _, c in au)
            pi = 0
            spent_p = 0.0
            spent_a = 0.0
            for f, c in au:
                # emit proj until its progress fraction catches attn's
                while pi < len(pu) and spent_p * at <= spent_a * pt:
                    pf, pc = pu[pi]
                    pf()
                    spent_p += pc
                    pi += 1
                f()
                spent_a += c
            while pi < len(pu):
                pu[pi][0]()
                pi += 1

        def merge_units(a, b):
            """Proportionally interleave two (closure, cost) unit lists."""
            out = []
            ta, tb = sum(c for _, c in a), sum(c for _, c in b)
            ai = bi = 0
            sa = sb = 0.0
            while ai < len(a) or bi < len(b):
                if bi >= len(b) or (ai < len(a) and sa * tb <= sb * ta):
                    out.append(a[ai]); sa += a[ai][1]; ai += 1
                else:
                    out.append(b[bi]); sb += b[bi][1]; bi += 1
            return out

        # slots: E_j = 2j+2 tiles; tile lt needs kt from window lt//4 and
        # qt chunk j. Q chunks are computed early (see table below) so
        # slots 4-7 can precompute S+exp 1-2 windows ahead, leaving the
        # final window as a dense bf16 O-stream instead of a serial
        # exp-bound phase.
        QCH = {0: [0, 1], 1: [2, 3, 4, 5], 2: [6, 7], 3: []}
        slot_fns = {}
        for j in range(8):
            o_tag = "k" if j == 7 else "o"
            out_eng = [nc.gpsimd, nc.sync, nc.scalar] if j == 6 else None
            slot_fns[j] = make_slot(j, o_tag=o_tag, out_eng=out_eng)

        def full(j):
            pre, post = slot_fns[j]
            E = 2 * j + 2
            order = ([E - 2, E - 1] + list(range(E - 2))) if E > 2 \
                else list(range(E))
            return post(order, order)

        weave(proj_units(0, QCH[0]), [])
        weave(proj_units(1, QCH[1]), full(0) + full(1))
        # windows 2-3: current slots plus S/exp precompute for later slots
        pre4, post4 = slot_fns[4]
        pre5, post5 = slot_fns[5]
        pre6, post6 = slot_fns[6]
        pre7, post7 = slot_fns[7]
        # precompute units are MERGED throughout the window (not appended)
        # so the s-tag PSUM rotation never outruns the scalar exp queue
        # without tensor filler in between. O-streams start with the
        # buffered (precomputed) tiles so they never wait on fresh exps.
        weave(proj_units(2, QCH[2]),
              merge_units(full(2) + full(3),
                          pre4(list(range(8))) + pre5(list(range(8)))))
        # slot 4 (E=10): fresh 8,9 are the diag pair; slot 5 (E=12): fresh
        # 8..11 with diag 10,11 first so the final tile is unmasked
        weave(proj_units(3, QCH[3]),
              merge_units(post4([8, 9], list(range(8)) + [8, 9])
                          + post5([10, 11, 8, 9],
                                  list(range(8)) + [10, 11, 8, 9]),
                          pre7(list(range(12))) + pre6(list(range(12)))))
        # last window: slot 7 completes (and writes out) first, then 6
        for fu, _ in post7([14, 15, 12, 13],
                           list(range(12)) + [14, 15, 12, 13]):
            fu()
        for fu, _ in post6([12, 13], list(range(12)) + [12, 13]):
            fu()

    nc.compile()
    return nc


def _prep_inputs(inputs, Wq, bq, Wk, bk, Wv, bv):
    import ml_dtypes
    bf16 = ml_dtypes.bfloat16

    def pack_w(W):
        return np.ascontiguousarray(
            W.reshape(8, 128, DK).transpose(1, 0, 2)).astype(bf16)

    wq_, wk_, wv_ = pack_w(Wq), pack_w(Wk), pack_w(Wv)
    wk0_ = np.ascontiguousarray(wk_[:, :, :128])
    wk1_ = np.ascontiguousarray(wk_[:, :, 128:])
    bqk = np.stack([bq[:128], bq[128:], bk[:128], bk[128:]],
                   axis=1).astype(np.float32)
    bvb = np.tile(bv[None, :], (128, 1)).astype(np.float32)

    # xt per (b, h): h=1 cores get each 256-col group's halves swapped so
    # the kernel's fixed [:, :, 0, :] parity read picks the odd tiles.
    # Side effect: h=1 query columns are pair-permuted within each slot;
    # colio then carries the ORIGINAL query index per position (for the
    # causal mask) and the host unswaps O/l columns before summing.
    perm = PERM
    xt_b = [np.ascontiguousarray(inputs[b].T).astype(bf16) for b in range(B)]
    xt_bh = {}
    for b in range(B):
        xt_bh[(b, 0)] = xt_b[b]
        sw = xt_b[b].reshape(DM, T // 256, 2, 128)[:, :, ::-1, :]
        xt_bh[(b, 1)] = np.ascontiguousarray(sw.reshape(DM, T))

    in_maps = []
    r = np.arange(128, dtype=np.float32)
    for core in range(8):
        b, h = core % B, core // B
        cvals = np.arange(QW, dtype=np.float32) if h == 0 \
            else perm.astype(np.float32)
        colio = np.tile(cvals[None, :], (128, 1))
        thr_np = np.empty((128, NLT), dtype=np.float32)
        for lt in range(NLT):
            j = lt // 2
            g = 2 * lt + h
            thr_np[:, lt] = 128 * g + r - QW * j
        cst = np.concatenate([bqk, bvb, colio, thr_np], axis=1)
        in_maps.append({
            "xt": xt_bh[(b, h)],
            "wq": wq_, "wk0": wk0_, "wk1": wk1_, "wv": wv_,
            "cst": np.ascontiguousarray(cst),
        })
    return in_maps


def kernel(inputs, Wq, bq, Wk, bk, Wv, bv):
    from concourse.bass_utils import run_bass_kernel_spmd

    if "nc" not in _CACHE:
        _CACHE["nc"] = _build_graph()
    nc = _CACHE["nc"]

    in_maps = _prep_inputs(
        np.asarray(inputs), np.asarray(Wq), np.asarray(bq), np.asarray(Wk),
        np.asarray(bk), np.asarray(Wv), np.asarray(bv))

    res = run_bass_kernel_spmd(nc, in_maps, core_ids=list(range(8)))
    _CACHE["last_results"] = res

    out = np.empty((B, T, DK), dtype=np.float32)
    for b in range(B):
        r0, r1 = res.results[b], res.results[b + 4]

        def unpack(r):
            # [slot, part, dvc, q] -> [slot, dv, q]
            o = np.asarray(r["o_t"]).astype(np.float32)
            return o.transpose(0, 2, 1, 3).reshape(NSLOT, DK, QW)

        o_sum = unpack(r0) + unpack(r1)[:, :, PERM]
        l_sum = np.asarray(r0["l_o"]) \
            + np.asarray(r1["l_o"])[:, :, PERM]
        for j in range(NSLOT):
            out[b, QW * j:QW * (j + 1), :] = (o_sum[j] / l_sum[j]).T
    return out


if __name__ == "__main__":
    import reference
    ins = {k: np.asarray(v) for k, v in reference.setup_inputs().items()}
    exp = np.asarray(reference.reference(**ins))
    act = kernel(**ins)
    err = np.linalg.norm(act - exp) / np.linalg.norm(exp)
    print("Relative error:", err)
